# revision 20
# baseline (speedup 1.0000x reference)
"""AffCoeffToMatrix TRN2 kernel (v6: fp16 planar I/O, quad-batched tables).

For each batch element (B = 2,000,000):
  R = rodrigues(rotat), U = rodrigues(scal_dir), D = exp(scal)
  M = R @ (U @ diag(D) @ U^T);  out = [M | trans]  -> [B, 3, 4] f32

Host marshals inputs to fp16 PLANAR layout (9 planes: r_xyz, u_xyz, s_xyz)
and reassembles the full [B,3,4] f32 output from the 9 fp16 M-planes the
device returns, inserting the trans column exactly.  Device HBM traffic is
36 B/elem (18 in + 18 out) vs 96 B/elem for f32 interleaved full I/O.

On-core: L = F*T elems/lane, processed as NQ quad-groups (chain width
FQ = 4F) of 2 build-blocks each (width F2 = 2F).  Transcendental chain is
quad-wide so the ACT table loads (natural_log_exp <-> trig) amortize:
2 loads per quad, 4 per sweep.

Math per rotation (v = axis vector, th2 = |v|^2):
  lg = ln(th2); th = e^{lg/2}; rt = e^{-lg/2 + ln sqrt2} = sqrt2/th
  sh = sin(th/2), ch = sin(u4/2 + pi/2) with u4 = th - 4pi*(th > pi)
  G = (sh*rt)*v        => G_i G_j = b v_i v_j   (b = 2 sh^2/th^2)
  c2 = 1 - 2 sh^2 = cos th,  C2 = sqrt2*ch  => C2*G_k = a v_k (a = sin/th)
  R = c2 I + G G^T + [C2 G]x
Scaling: W = U diag(e^{s/2}), S = W W^T (6 unique), M = R @ S.

Engines (v1 cost model): ACT squares+transcendentals, DVE fp16 2x tensor
ops + 4x tensor-scalar, Pool assembly adds, SP all DMAs.
"""
import math
import sys

for _p in ("/opt/trn_rl_repo", "/root/.axon_site/_ro/trn_rl_repo"):
    if _p not in sys.path:
        sys.path.append(_p)

import numpy as np

import concourse.bass as bass
import concourse.mybir as mybir
import concourse.tile as tile

F32 = mybir.dt.float32
F16 = mybir.dt.float16
AF = mybir.ActivationFunctionType
OP = mybir.AluOpType
PI = math.pi

# ---- hardcoded problem geometry ----
B = 2_000_000
N_CORES = 8
P = 128
F = 246            # base tile width (2F chunks = 984B >= 512B DMA full rate)
F2 = 2 * F         # group width (chains and builds)
NQ = 4             # groups per sweep
L = F2 * NQ        # elems per partition lane (1968)
E = P * L          # elems per core (251904)
BPAD = N_CORES * E


def _split_multi_waits(nc, limit=1, drain_limit=0):
    """This container's walrus cannot encode >1 sync-wait per instruction
    (Drain: none at all). Spill extras onto same-engine NOPs."""
    for b in nc.main_func.blocks:
        new = []
        for ins in b.instructions:
            si = getattr(ins, "sync_info", None)
            waits = list(si.on_wait) if (si is not None and si.on_wait) else []
            lim = drain_limit if isinstance(ins, mybir.InstDrain) else limit
            if len(waits) > lim:
                keep, spill = waits[:lim], waits[lim:]
                for w in spill:
                    nop = mybir.InstNoOp(
                        name=nc.get_next_instruction_name(),
                        sync_info=mybir.SyncInfo(on_wait=[w], on_update=[]),
                        bass_nofuse=True,
                        engine=ins.engine,
                    )
                    nc.register_instruction(nop)
                    new.append(nop)
                ins.sync_info = mybir.SyncInfo(
                    on_wait=keep, on_update=list(si.on_update or [])
                )
            new.append(ins)
        b.instructions[:] = new


def build_module():
    nc = bass.Bass()
    in9 = nc.dram_tensor("in9", [9, E], F16, kind="ExternalInput")
    out9 = nc.dram_tensor("out9", [9, E], F16, kind="ExternalOutput")

    FQ = 4 * F          # chain/quad width (984)
    NQ = 2              # quads per sweep
    NB = 4              # build blocks per quad (width F)
    F1 = F

    vin = in9[:].rearrange("k (q p f) -> q p k f", q=NQ, p=P)     # [P,9,FQ]
    vout = out9[:].rearrange("k (q p f) -> q p k f", q=NQ, p=P)      # [P,9,FQ]

    with tile.TileContext(nc) as tc:
        with (
            tc.tile_pool(name="pin", bufs=2) as pin,      # in36 quad
            tc.tile_pool(name="pth2", bufs=1) as pth2,    # th2 f32 (Pool writes)
            tc.tile_pool(name="ppsum", bufs=1, space="PSUM") as ppsum,  # lg/th/m4
            tc.tile_pool(name="pch", bufs=1) as pch,      # chain fp16 transients
            tc.tile_pool(name="pcf", bufs=2) as pcf,      # t2, C2, c2 survivors
            tc.tile_pool(name="pe3", bufs=1) as pe3,      # e3
            tc.tile_pool(name="psq", bufs=1) as psq,      # squares scratch
            tc.tile_pool(name="pbld", bufs=2) as pbld,    # G/dG/av/p6 (F1 blocks)
            tc.tile_pool(name="pru", bufs=2) as pru,      # RU18
            tc.tile_pool(name="pmat", bufs=2) as pmat,    # W9/sqW/S9/pp (+mp reuse)
            tc.tile_pool(name="pms", bufs=1) as pms,      # ms
            tc.tile_pool(name="pout", bufs=2) as pout,    # out per block-pair
            tc.tile_pool(name="pc", bufs=1) as pc,
        ):
            # const bias tiles + dummy Ln to warm the natural_log_exp table
            # during the first DMA
            lnr2 = pc.tile([P, 1], F32, tag="lnr2")
            nc.vector.memset(lnr2[:], 0.5 * math.log(2.0))
            pi2 = pc.tile([P, 1], F32, tag="pi2")
            nc.vector.memset(pi2[:], PI / 2)
            warm1 = pc.tile([P, 1], F32, tag="warm1")
            nc.scalar.activation(warm1[:], pi2[:], AF.Ln)

            def chain(q):
                in36 = pin.tile([P, 9 * FQ], F16, tag="in36", name="in36")
                v36 = in36[:].rearrange("p (k f) -> p k f", k=9)
                HW = 2 * F1  # half-quad width
                for h in (0, 1):
                    sl = slice(h * HW, (h + 1) * HW)
                    nc.sync.dma_start(out=v36[:, :, sl], in_=vin[q][:, :, sl])

                # th2 quad [P, 2rot, FQ] f32, squares per half-quad
                th2 = pth2.tile([P, 2 * FQ], F32, tag="th2", name="th2")
                th2v = th2[:].rearrange("p (r f) -> p r f", r=2)
                for h in (0, 1):
                    sl = slice(h * HW, (h + 1) * HW)
                    sq = psq.tile([P, 12 * F1], F16, tag="sq", name="sq")
                    sqv = sq[:].rearrange("p (c f) -> p c f", c=6)
                    nc.scalar.activation(sqv, v36[:, 0:6, sl], AF.Square)
                    tmp = psq.tile([P, 4 * F1], F16, tag="tmp", name="tmp")
                    tmpv = tmp[:].rearrange("p (r f) -> p r f", r=2)
                    nc.gpsimd.tensor_add(tmpv, sqv[:, 0:4:3, :], sqv[:, 1:5:3, :])
                    nc.gpsimd.tensor_add(th2v[:, :, sl], tmpv, sqv[:, 2:6:3, :])

                # chain tiles (quad-wide), ops per half-quad for latency
                lg = ppsum.tile([P, 2 * FQ], F32, tag="lg", name="lg")
                th = pch.tile([P, 2 * FQ], F16, tag="th", name="th")
                rt = pch.tile([P, 2 * FQ], F16, tag="rt", name="rt")
                e3 = pe3.tile([P, 3 * FQ], F16, tag="e3", name="e3")
                e3v = e3[:].rearrange("p (c f) -> p c f", c=3)
                m4 = pch.tile([P, 2 * FQ], F16, tag="m4", name="m4")
                u4 = pch.tile([P, 2 * FQ], F16, tag="u4", name="u4")
                sh = pch.tile([P, 2 * FQ], F16, tag="sh", name="sh")
                ch = pch.tile([P, 2 * FQ], F16, tag="ch", name="ch")
                shsq = pch.tile([P, 2 * FQ], F16, tag="shsq", name="shsq")
                t2 = pcf.tile([P, 2 * FQ], F16, tag="t2", name="t2")
                C2 = pcf.tile([P, 2 * FQ], F16, tag="C2", name="C2")
                c2 = pcf.tile([P, 2 * FQ], F16, tag="c2", name="c2")

                def hs2(ap, h):  # [P, 2, FQ]-flat half-slice as 2D views
                    v = ap[:].rearrange("p (r f) -> p r f", r=2)
                    return v[:, :, h * HW : (h + 1) * HW]

                # natural_log_exp phase (both halves)
                for h in (0, 1):
                    sl = slice(h * HW, (h + 1) * HW)
                    nc.scalar.activation(hs2(lg, h), hs2(th2, h), AF.Ln)
                    nc.scalar.activation(hs2(th, h), hs2(lg, h), AF.Exp, scale=0.5)
                    nc.scalar.activation(
                        hs2(rt, h), hs2(lg, h), AF.Exp, scale=-0.5, bias=lnr2[:]
                    )
                    nc.scalar.activation(
                        e3v[:, :, sl], v36[:, 6:9, sl], AF.Exp, scale=0.5
                    )
                    nc.vector.tensor_scalar(
                        hs2(m4, h), hs2(th, h), PI, -4 * PI, OP.is_gt, OP.mult
                    )
                    nc.gpsimd.tensor_add(hs2(u4, h), hs2(m4, h), hs2(th, h))
                # trig phase (both halves)
                for h in (0, 1):
                    nc.scalar.activation(hs2(sh, h), hs2(th, h), AF.Sin, scale=0.5)
                    nc.scalar.activation(
                        hs2(ch, h), hs2(u4, h), AF.Sin, scale=0.5, bias=pi2[:]
                    )
                    nc.scalar.activation(hs2(shsq, h), hs2(sh, h), AF.Square)
                    nc.vector.tensor_mul(hs2(t2, h), hs2(sh, h), hs2(rt, h))
                    nc.vector.tensor_scalar(
                        hs2(C2, h), hs2(ch, h), math.sqrt(2.0), None, OP.mult
                    )
                    nc.vector.tensor_scalar(
                        hs2(c2, h), hs2(shsq, h), -2.0, 1.0, OP.mult, OP.add
                    )
                return {
                    "v36": v36,
                    "t2": t2[:].rearrange("p (r f) -> p r f", r=2),
                    "C2": C2[:].rearrange("p (r f) -> p r f", r=2),
                    "c2": c2[:].rearrange("p (r f) -> p r f", r=2),
                    "e3": e3v,
                }

            def build(st, b, ot, osl, last):
                """One F1-wide build block; writes M into ot[:, :, osl]."""
                sl = slice(b * F1, (b + 1) * F1)
                vv = st["v36"][:, 0:6, sl].rearrange("p (r c) f -> p r c f", r=2)
                t2s = st["t2"][:, :, sl]
                C2s = st["C2"][:, :, sl].unsqueeze(2)
                c2s = st["c2"][:, :, sl]
                e3s = st["e3"][:, :, sl]

                # G = t2 * v
                G = pbld.tile([P, 6 * F1], F16, tag="G", name="G")
                Gv = G[:].rearrange("p (r c f) -> p r c f", r=2, c=3)
                nc.vector.tensor_mul(
                    Gv, t2s.unsqueeze(2).to_broadcast((P, 2, 3, F1)), vv
                )
                # dG = G^2 (ACT, table-free)
                dG = pbld.tile([P, 6 * F1], F16, tag="dG", name="dG")
                dGv = dG[:].rearrange("p (r c f) -> p r c f", r=2, c=3)
                nc.scalar.activation(dGv, Gv, AF.Square)
                # av planes in (z, x, y) order: av = C2 * G_perm
                av = pbld.tile([P, 6 * F1], F16, tag="av", name="av")
                avv = av[:].rearrange("p (r c f) -> p r c f", r=2, c=3)
                nc.vector.tensor_mul(
                    avv[:, :, 0:1, :],
                    C2s.to_broadcast((P, 2, 1, F1)),
                    Gv[:, :, 2:3, :],
                )
                nc.vector.tensor_mul(
                    avv[:, :, 1:3, :],
                    C2s.to_broadcast((P, 2, 2, F1)),
                    Gv[:, :, 0:2, :],
                )
                # p6 = (G0G1, G1G2, G2G0)
                p6 = pbld.tile([P, 6 * F1], F16, tag="p6", name="p6")
                p6v = p6[:].rearrange("p (r c f) -> p r c f", r=2, c=3)
                nc.vector.tensor_mul(
                    p6v[:, :, 0:2, :], Gv[:, :, 0:2, :], Gv[:, :, 1:3, :]
                )
                nc.vector.tensor_mul(
                    p6v[:, :, 2:3, :], Gv[:, :, 2:3, :], Gv[:, :, 0:1, :]
                )

                # RU18 assembly (Pool): R = c2 I + GG^T + [C2 G]x
                RU18 = pru.tile([P, 18 * F1], F16, tag="RU18", name="RU18")
                ruv = RU18[:].rearrange("p (r k f) -> p r k f", r=2, k=9)
                c2b = c2s.unsqueeze(2).to_broadcast((P, 2, 3, F1))
                nc.gpsimd.tensor_add(ruv[:, :, 0:9:4, :], dGv, c2b)
                nc.gpsimd.tensor_add(
                    ruv[:, :, 3:8:4, :], p6v[:, :, 0:2, :], avv[:, :, 0:2, :]
                )
                nc.gpsimd.tensor_add(
                    ruv[:, :, 2, :], p6v[:, :, 2, :], avv[:, :, 2, :]
                )
                nc.gpsimd.tensor_sub(
                    ruv[:, :, 1:6:4, :], p6v[:, :, 0:2, :], avv[:, :, 0:2, :]
                )
                nc.gpsimd.tensor_sub(
                    ruv[:, :, 6, :], p6v[:, :, 2, :], avv[:, :, 2, :]
                )

                R9v = RU18[:, : 9 * F1].rearrange("p (k f) -> p k f", k=9)
                U9v = RU18[:, 9 * F1 :].rearrange("p (i k f) -> p i k f", i=3, k=3)

                # W = U diag(e) (DVE), sqW (ACT)
                W9 = pmat.tile([P, 9 * F1], F16, tag="W9", name="W9")
                W9v4 = W9[:].rearrange("p (i k f) -> p i k f", i=3, k=3)
                nc.vector.tensor_mul(
                    W9v4, U9v, e3s.unsqueeze(1).to_broadcast((P, 3, 3, F1))
                )
                sqW = pmat.tile([P, 9 * F1], F16, tag="sqW", name="sqW")
                nc.scalar.activation(sqW[:], W9[:], AF.Square)
                sqWv = sqW[:].rearrange("p (i k f) -> p i k f", i=3, k=3)

                # S unique-6: S00@0 S01@1 S02@2 S11@3 S12@5 S22@8
                S9 = pmat.tile([P, 9 * F1], F16, tag="S9", name="S9")
                S9v = S9[:].rearrange("p (k f) -> p k f", k=9)
                sdt = psq.tile([P, 3 * F1], F16, tag="sdt", name="sdt")
                sdtv = sdt[:].rearrange("p (c f) -> p c f", c=3)
                nc.gpsimd.tensor_add(sdtv, sqWv[:, :, 0, :], sqWv[:, :, 1, :])
                nc.gpsimd.tensor_add(
                    S9v[:, 0:4:3, :], sdtv[:, 0:2, :], sqWv[:, 0:2, 2, :]
                )
                nc.gpsimd.tensor_add(S9v[:, 8, :], sdtv[:, 2, :], sqWv[:, 2, 2, :])
                # pp: row-pair products (01, 02, 12)
                pp = pmat.tile([P, 9 * F1], F16, tag="pp", name="pp")
                ppv = pp[:].rearrange("p (g k f) -> p g k f", g=3, k=3)
                nc.vector.tensor_mul(
                    ppv[:, 0:2, :, :],
                    W9v4[:, 0, :, :].unsqueeze(1).to_broadcast((P, 2, 3, F1)),
                    W9v4[:, 1:3, :, :],
                )
                nc.vector.tensor_mul(
                    ppv[:, 2, :, :], W9v4[:, 1, :, :], W9v4[:, 2, :, :]
                )
                q3 = psq.tile([P, 3 * F1], F16, tag="q3", name="q3")
                q3v = q3[:].rearrange("p (g f) -> p g f", g=3)
                nc.gpsimd.tensor_add(q3v, ppv[:, :, 0, :], ppv[:, :, 1, :])
                nc.gpsimd.tensor_add(
                    S9v[:, 1:3, :], q3v[:, 0:2, :], ppv[:, 0:2, 2, :]
                )
                nc.gpsimd.tensor_add(S9v[:, 5, :], q3v[:, 2, :], ppv[:, 2, 2, :])

                # M = R @ S (DVE muls, Pool final add into out tile)
                srows = [S9v[:, 0:3, :], S9v[:, 1:7:2, :], S9v[:, 2:9:3, :]]
                otv = ot[:].rearrange("p (i j f) -> p i j f", i=3, j=3)[
                    :, :, :, osl
                ]

                def colb(k):
                    return (
                        R9v[:, k : k + 7 : 3, :]
                        .unsqueeze(2)
                        .to_broadcast((P, 3, 3, F1))
                    )

                def rowb(sr):
                    return sr.unsqueeze(1).to_broadcast((P, 3, 3, F1))

                mp1 = pmat.tile([P, 9 * F1], F16, tag="pp", name="mp1")
                mp1v = mp1[:].rearrange("p (i j f) -> p i j f", i=3, j=3)
                nc.vector.tensor_mul(mp1v, colb(0), rowb(srows[0]))
                mp2 = pmat.tile([P, 9 * F1], F16, tag="sqW", name="mp2")
                mp2v = mp2[:].rearrange("p (i j f) -> p i j f", i=3, j=3)
                nc.vector.tensor_mul(mp2v, colb(1), rowb(srows[1]))
                ms = pms.tile([P, 9 * F1], F16, tag="ms", name="ms")
                msv = ms[:].rearrange("p (i j f) -> p i j f", i=3, j=3)
                (nc.vector if b % 2 == 0 else nc.gpsimd).tensor_add(
                    msv, mp1v, mp2v
                )
                mp3 = pmat.tile([P, 9 * F1], F16, tag="W9", name="mp3")
                mp3v = mp3[:].rearrange("p (i j f) -> p i j f", i=3, j=3)
                nc.vector.tensor_mul(mp3v, colb(2), rowb(srows[2]))
                nc.gpsimd.tensor_add(otv, msv, mp3v)

            for q in range(NQ):
                st = chain(q)
                for pr in range(2):  # block-pairs per quad
                    ot = pout.tile([P, 18 * F1], F16, tag="ot", name="ot")
                    for j in range(2):
                        b = 2 * pr + j
                        osl = slice(j * F1, (j + 1) * F1)
                        build(st, b, ot, osl, last=(q == NQ - 1 and b == 3))
                    osl2 = slice(pr * 2 * F1, (pr + 1) * 2 * F1)
                    nc.sync.dma_start(
                        out=vout[q][:, :, osl2],
                        in_=ot[:].rearrange("p (k f) -> p k f", k=9),
                    )

    _split_multi_waits(nc)
    return nc


# ----------------------------------------------------------------------------
# host-side execution
# ----------------------------------------------------------------------------
_CACHE = {}


def _get_runner():
    if "runner" in _CACHE:
        return _CACHE["runner"]
    import jax
    from jax.sharding import Mesh, PartitionSpec
    from jax.experimental.shard_map import shard_map
    from concourse.bass2jax import (
        _bass_exec_p,
        install_neuronx_cc_hook,
        partition_id_tensor,
    )

    nc = build_module()
    install_neuronx_cc_hook()
    partition_name = nc.partition_id_tensor.name if nc.partition_id_tensor else None
    in_names, out_names, out_avals, zero_outs = [], [], [], []
    for alloc in nc.m.functions[0].allocations:
        if not isinstance(alloc, mybir.MemoryLocationSet):
            continue
        name = alloc.memorylocations[0].name
        if alloc.kind == "ExternalInput":
            if name != partition_name:
                in_names.append(name)
        elif alloc.kind == "ExternalOutput":
            shape = tuple(alloc.tensor_shape)
            dtype = mybir.dt.np(alloc.dtype)
            out_names.append(name)
            out_avals.append(jax.core.ShapedArray(shape, dtype))
            zero_outs.append(np.zeros(shape, dtype))
    n_params = len(in_names)
    all_in_names = in_names + out_names + (
        [partition_name] if partition_name else []
    )

    def _body(*args):
        operands = list(args)
        if partition_name is not None:
            operands.append(partition_id_tensor())
        outs = _bass_exec_p.bind(
            *operands,
            out_avals=tuple(out_avals),
            in_names=tuple(all_in_names),
            out_names=tuple(out_names),
            lowering_input_output_aliases=(),
            sim_require_finite=True,
            sim_require_nnan=True,
            nc=nc,
        )
        return tuple(outs)

    devices = jax.devices()[:N_CORES]
    mesh = Mesh(np.asarray(devices), ("core",))
    n_outs = len(out_names)
    jf = jax.jit(
        shard_map(
            _body,
            mesh=mesh,
            in_specs=(PartitionSpec("core"),) * (n_params + n_outs),
            out_specs=(PartitionSpec("core"),) * n_outs,
            check_rep=False,
        ),
        donate_argnums=tuple(range(n_params, n_params + n_outs)),
        keep_unused=True,
    )
    _CACHE["runner"] = (jf, in_names, out_names, zero_outs)
    return _CACHE["runner"]


def kernel(trans, rotat, scal_dir, scal):
    jf, in_names, out_names, zero_outs = _get_runner()
    assert in_names == ["in9"], in_names

    # fp16 planar marshalling: planes (rx ry rz ux uy uz sx sy sz) per core
    a = np.empty((N_CORES, 9, E), dtype=np.float16)
    buf = np.ones((BPAD, 3), dtype=np.float16)
    for i, src in enumerate((rotat, scal_dir, scal)):
        buf[:B] = src
        if i > 0:
            buf[B:] = 1.0
        a[:, 3 * i : 3 * i + 3, :] = buf.reshape(N_CORES, E, 3).transpose(0, 2, 1)

    in9_host = a.reshape(N_CORES * 9, E)
    zeros = [
        np.zeros((N_CORES * z.shape[0], *z.shape[1:]), z.dtype) for z in zero_outs
    ]
    outs = jf(in9_host, *zeros)
    o = np.asarray(outs[0]).reshape(N_CORES, 9, E)
    m = o.transpose(0, 2, 1).reshape(BPAD, 9)[:B].astype(np.float32)
    out = np.empty((B, 3, 4), dtype=np.float32)
    out[:, :, :3] = m.reshape(B, 3, 3)
    out[:, :, 3] = trans
    return out


if __name__ == "__main__":
    rng = np.random.default_rng(0)
    ins = {
        "trans": rng.normal(size=(B, 3)).astype(np.float32),
        "rotat": rng.normal(size=(B, 3)).astype(np.float32),
        "scal_dir": rng.normal(size=(B, 3)).astype(np.float32),
        "scal": rng.normal(size=(B, 3)).astype(np.float32),
    }
    out = kernel(**ins)
    print(out.shape, out.dtype)


# revision 21
# speedup vs baseline: 1.0246x; 1.0246x over previous
"""AffCoeffToMatrix TRN2 kernel (v6: fp16 planar I/O, quad-batched tables).

For each batch element (B = 2,000,000):
  R = rodrigues(rotat), U = rodrigues(scal_dir), D = exp(scal)
  M = R @ (U @ diag(D) @ U^T);  out = [M | trans]  -> [B, 3, 4] f32

Host marshals inputs to fp16 PLANAR layout (9 planes: r_xyz, u_xyz, s_xyz)
and reassembles the full [B,3,4] f32 output from the 9 fp16 M-planes the
device returns, inserting the trans column exactly.  Device HBM traffic is
36 B/elem (18 in + 18 out) vs 96 B/elem for f32 interleaved full I/O.

On-core: L = F*T elems/lane, processed as NQ quad-groups (chain width
FQ = 4F) of 2 build-blocks each (width F2 = 2F).  Transcendental chain is
quad-wide so the ACT table loads (natural_log_exp <-> trig) amortize:
2 loads per quad, 4 per sweep.

Math per rotation (v = axis vector, th2 = |v|^2):
  lg = ln(th2); th = e^{lg/2}; rt = e^{-lg/2 + ln sqrt2} = sqrt2/th
  sh = sin(th/2), ch = sin(u4/2 + pi/2) with u4 = th - 4pi*(th > pi)
  G = (sh*rt)*v        => G_i G_j = b v_i v_j   (b = 2 sh^2/th^2)
  c2 = 1 - 2 sh^2 = cos th,  C2 = sqrt2*ch  => C2*G_k = a v_k (a = sin/th)
  R = c2 I + G G^T + [C2 G]x
Scaling: W = U diag(e^{s/2}), S = W W^T (6 unique), M = R @ S.

Engines (v1 cost model): ACT squares+transcendentals, DVE fp16 2x tensor
ops + 4x tensor-scalar, Pool assembly adds, SP all DMAs.
"""
import math
import sys

for _p in ("/opt/trn_rl_repo", "/root/.axon_site/_ro/trn_rl_repo"):
    if _p not in sys.path:
        sys.path.append(_p)

import numpy as np

import concourse.bass as bass
import concourse.mybir as mybir
import concourse.tile as tile

F32 = mybir.dt.float32
F16 = mybir.dt.float16
AF = mybir.ActivationFunctionType
OP = mybir.AluOpType
PI = math.pi

# ---- hardcoded problem geometry ----
B = 2_000_000
N_CORES = 8
P = 128
F = 246            # base tile width (2F chunks = 984B >= 512B DMA full rate)
F2 = 2 * F         # group width (chains and builds)
NQ = 4             # groups per sweep
L = F2 * NQ        # elems per partition lane (1968)
E = P * L          # elems per core (251904)
BPAD = N_CORES * E


def _split_multi_waits(nc, limit=1, drain_limit=0):
    """This container's walrus cannot encode >1 sync-wait per instruction
    (Drain: none at all). Spill extras onto same-engine NOPs."""
    for b in nc.main_func.blocks:
        new = []
        for ins in b.instructions:
            si = getattr(ins, "sync_info", None)
            waits = list(si.on_wait) if (si is not None and si.on_wait) else []
            lim = drain_limit if isinstance(ins, mybir.InstDrain) else limit
            if len(waits) > lim:
                keep, spill = waits[:lim], waits[lim:]
                for w in spill:
                    nop = mybir.InstNoOp(
                        name=nc.get_next_instruction_name(),
                        sync_info=mybir.SyncInfo(on_wait=[w], on_update=[]),
                        bass_nofuse=True,
                        engine=ins.engine,
                    )
                    nc.register_instruction(nop)
                    new.append(nop)
                ins.sync_info = mybir.SyncInfo(
                    on_wait=keep, on_update=list(si.on_update or [])
                )
            new.append(ins)
        b.instructions[:] = new


def build_module():
    nc = bass.Bass()
    in9 = nc.dram_tensor("in9", [9, E], F16, kind="ExternalInput")
    out9 = nc.dram_tensor("out9", [9, E], F16, kind="ExternalOutput")

    FQ = 4 * F          # chain/quad width (984)
    NQ = 2              # quads per sweep
    NB = 4              # build blocks per quad (width F)
    F1 = F

    vin = in9[:].rearrange("k (q p f) -> q p k f", q=NQ, p=P)     # [P,9,FQ]
    vout = out9[:].rearrange("k (q p f) -> q p k f", q=NQ, p=P)      # [P,9,FQ]

    with tile.TileContext(nc) as tc:
        with (
            tc.tile_pool(name="pin", bufs=2) as pin,      # in36 quad
            tc.tile_pool(name="pth2", bufs=1) as pth2,    # th2 f32 (Pool writes)
            tc.tile_pool(name="ppsum", bufs=1, space="PSUM") as ppsum,  # lg/th/m4
            tc.tile_pool(name="pch", bufs=1) as pch,      # chain fp16 transients
            tc.tile_pool(name="pcf", bufs=2) as pcf,      # t2, C2, c2 survivors
            tc.tile_pool(name="pe3", bufs=1) as pe3,      # e3
            tc.tile_pool(name="psq", bufs=1) as psq,      # squares scratch
            tc.tile_pool(name="pbld", bufs=2) as pbld,    # G/dG/av/p6 (F1 blocks)
            tc.tile_pool(name="pru", bufs=2) as pru,      # RU18
            tc.tile_pool(name="pmat", bufs=2) as pmat,    # W9/sqW/S9/pp (+mp reuse)
            tc.tile_pool(name="pms", bufs=1) as pms,      # ms
            tc.tile_pool(name="pout", bufs=2) as pout,    # out per block-pair
            tc.tile_pool(name="pc", bufs=1) as pc,
        ):
            # const bias tiles + dummy Ln to warm the natural_log_exp table
            # during the first DMA
            lnr2 = pc.tile([P, 1], F32, tag="lnr2")
            nc.vector.memset(lnr2[:], 0.5 * math.log(2.0))
            pi2 = pc.tile([P, 1], F32, tag="pi2")
            nc.vector.memset(pi2[:], PI / 2)
            warm1 = pc.tile([P, 1], F32, tag="warm1")
            nc.scalar.activation(warm1[:], pi2[:], AF.Ln)

            def chain(q):
                in36 = pin.tile([P, 9 * FQ], F16, tag="in36", name="in36")
                v36 = in36[:].rearrange("p (k f) -> p k f", k=9)
                HW = 2 * F1  # half-quad width
                for h in (0, 1):
                    sl = slice(h * HW, (h + 1) * HW)
                    nc.sync.dma_start(out=v36[:, :, sl], in_=vin[q][:, :, sl])

                # th2 quad [P, 2rot, FQ] f32, squares per half-quad
                th2 = pth2.tile([P, 2 * FQ], F32, tag="th2", name="th2")
                th2v = th2[:].rearrange("p (r f) -> p r f", r=2)
                for h in (0, 1):
                    sl = slice(h * HW, (h + 1) * HW)
                    sq = psq.tile([P, 12 * F1], F16, tag="sq", name="sq")
                    sqv = sq[:].rearrange("p (c f) -> p c f", c=6)
                    nc.scalar.activation(sqv, v36[:, 0:6, sl], AF.Square)
                    tmp = psq.tile([P, 4 * F1], F16, tag="tmp", name="tmp")
                    tmpv = tmp[:].rearrange("p (r f) -> p r f", r=2)
                    nc.gpsimd.tensor_add(tmpv, sqv[:, 0:4:3, :], sqv[:, 1:5:3, :])
                    nc.gpsimd.tensor_add(th2v[:, :, sl], tmpv, sqv[:, 2:6:3, :])

                # chain tiles (quad-wide), ops per half-quad for latency
                lg = ppsum.tile([P, 2 * FQ], F32, tag="lg", name="lg")
                th = pch.tile([P, 2 * FQ], F16, tag="th", name="th")
                rt = pch.tile([P, 2 * FQ], F16, tag="rt", name="rt")
                e3 = pe3.tile([P, 3 * FQ], F16, tag="e3", name="e3")
                e3v = e3[:].rearrange("p (c f) -> p c f", c=3)
                m4 = pch.tile([P, 2 * FQ], F16, tag="m4", name="m4")
                u4 = pch.tile([P, 2 * FQ], F16, tag="u4", name="u4")
                sh = pch.tile([P, 2 * FQ], F16, tag="sh", name="sh")
                ch = pch.tile([P, 2 * FQ], F16, tag="ch", name="ch")
                shsq = pch.tile([P, 2 * FQ], F16, tag="shsq", name="shsq")
                t2 = pcf.tile([P, 2 * FQ], F16, tag="t2", name="t2")
                C2 = pcf.tile([P, 2 * FQ], F16, tag="C2", name="C2")
                c2 = pcf.tile([P, 2 * FQ], F16, tag="c2", name="c2")

                def hs2(ap, h):  # [P, 2, FQ]-flat half-slice as 2D views
                    v = ap[:].rearrange("p (r f) -> p r f", r=2)
                    return v[:, :, h * HW : (h + 1) * HW]

                # natural_log_exp phase (both halves)
                for h in (0, 1):
                    sl = slice(h * HW, (h + 1) * HW)
                    nc.scalar.activation(hs2(lg, h), hs2(th2, h), AF.Ln)
                    nc.scalar.activation(hs2(th, h), hs2(lg, h), AF.Exp, scale=0.5)
                    nc.scalar.activation(
                        hs2(rt, h), hs2(lg, h), AF.Exp, scale=-0.5, bias=lnr2[:]
                    )
                    nc.scalar.activation(
                        e3v[:, :, sl], v36[:, 6:9, sl], AF.Exp, scale=0.5
                    )
                    nc.vector.tensor_scalar(
                        hs2(m4, h), hs2(th, h), PI, -4 * PI, OP.is_gt, OP.mult
                    )
                    nc.gpsimd.tensor_add(hs2(u4, h), hs2(m4, h), hs2(th, h))
                # trig phase (quad-wide: real dep on both th/u4 halves
                # pins it after the whole natlog phase -> no table thrash)
                nc.scalar.activation(sh[:], th[:], AF.Sin, scale=0.5)
                nc.scalar.activation(ch[:], u4[:], AF.Sin, scale=0.5, bias=pi2[:])
                nc.scalar.activation(shsq[:], sh[:], AF.Square)
                nc.vector.tensor_mul(t2[:], sh[:], rt[:])
                nc.vector.tensor_scalar(C2[:], ch[:], math.sqrt(2.0), None, OP.mult)
                nc.vector.tensor_scalar(c2[:], shsq[:], -2.0, 1.0, OP.mult, OP.add)
                return {
                    "v36": v36,
                    "t2": t2[:].rearrange("p (r f) -> p r f", r=2),
                    "C2": C2[:].rearrange("p (r f) -> p r f", r=2),
                    "c2": c2[:].rearrange("p (r f) -> p r f", r=2),
                    "e3": e3v,
                }

            def build(st, b, ot, osl, last):
                """One F1-wide build block; writes M into ot[:, :, osl]."""
                sl = slice(b * F1, (b + 1) * F1)
                vv = st["v36"][:, 0:6, sl].rearrange("p (r c) f -> p r c f", r=2)
                t2s = st["t2"][:, :, sl]
                C2s = st["C2"][:, :, sl].unsqueeze(2)
                c2s = st["c2"][:, :, sl]
                e3s = st["e3"][:, :, sl]

                # G = t2 * v
                G = pbld.tile([P, 6 * F1], F16, tag="G", name="G")
                Gv = G[:].rearrange("p (r c f) -> p r c f", r=2, c=3)
                nc.vector.tensor_mul(
                    Gv, t2s.unsqueeze(2).to_broadcast((P, 2, 3, F1)), vv
                )
                # dG = G^2 (ACT, table-free)
                dG = pbld.tile([P, 6 * F1], F16, tag="dG", name="dG")
                dGv = dG[:].rearrange("p (r c f) -> p r c f", r=2, c=3)
                nc.scalar.activation(dGv, Gv, AF.Square)
                # av planes in (z, x, y) order: av = C2 * G_perm
                av = pbld.tile([P, 6 * F1], F16, tag="av", name="av")
                avv = av[:].rearrange("p (r c f) -> p r c f", r=2, c=3)
                nc.vector.tensor_mul(
                    avv[:, :, 0:1, :],
                    C2s.to_broadcast((P, 2, 1, F1)),
                    Gv[:, :, 2:3, :],
                )
                nc.vector.tensor_mul(
                    avv[:, :, 1:3, :],
                    C2s.to_broadcast((P, 2, 2, F1)),
                    Gv[:, :, 0:2, :],
                )
                # p6 = (G0G1, G1G2, G2G0)
                p6 = pbld.tile([P, 6 * F1], F16, tag="p6", name="p6")
                p6v = p6[:].rearrange("p (r c f) -> p r c f", r=2, c=3)
                nc.vector.tensor_mul(
                    p6v[:, :, 0:2, :], Gv[:, :, 0:2, :], Gv[:, :, 1:3, :]
                )
                nc.vector.tensor_mul(
                    p6v[:, :, 2:3, :], Gv[:, :, 2:3, :], Gv[:, :, 0:1, :]
                )

                # RU18 assembly (Pool): R = c2 I + GG^T + [C2 G]x
                RU18 = pru.tile([P, 18 * F1], F16, tag="RU18", name="RU18")
                ruv = RU18[:].rearrange("p (r k f) -> p r k f", r=2, k=9)
                c2b = c2s.unsqueeze(2).to_broadcast((P, 2, 3, F1))
                nc.gpsimd.tensor_add(ruv[:, :, 0:9:4, :], dGv, c2b)
                nc.gpsimd.tensor_add(
                    ruv[:, :, 3:8:4, :], p6v[:, :, 0:2, :], avv[:, :, 0:2, :]
                )
                nc.gpsimd.tensor_add(
                    ruv[:, :, 2, :], p6v[:, :, 2, :], avv[:, :, 2, :]
                )
                nc.gpsimd.tensor_sub(
                    ruv[:, :, 1:6:4, :], p6v[:, :, 0:2, :], avv[:, :, 0:2, :]
                )
                nc.gpsimd.tensor_sub(
                    ruv[:, :, 6, :], p6v[:, :, 2, :], avv[:, :, 2, :]
                )

                R9v = RU18[:, : 9 * F1].rearrange("p (k f) -> p k f", k=9)
                U9v = RU18[:, 9 * F1 :].rearrange("p (i k f) -> p i k f", i=3, k=3)

                # W = U diag(e) (DVE), sqW (ACT)
                W9 = pmat.tile([P, 9 * F1], F16, tag="W9", name="W9")
                W9v4 = W9[:].rearrange("p (i k f) -> p i k f", i=3, k=3)
                nc.vector.tensor_mul(
                    W9v4, U9v, e3s.unsqueeze(1).to_broadcast((P, 3, 3, F1))
                )
                sqW = pmat.tile([P, 9 * F1], F16, tag="sqW", name="sqW")
                nc.scalar.activation(sqW[:], W9[:], AF.Square)
                sqWv = sqW[:].rearrange("p (i k f) -> p i k f", i=3, k=3)

                # S unique-6: S00@0 S01@1 S02@2 S11@3 S12@5 S22@8
                S9 = pmat.tile([P, 9 * F1], F16, tag="S9", name="S9")
                S9v = S9[:].rearrange("p (k f) -> p k f", k=9)
                sdt = psq.tile([P, 3 * F1], F16, tag="sdt", name="sdt")
                sdtv = sdt[:].rearrange("p (c f) -> p c f", c=3)
                nc.gpsimd.tensor_add(sdtv, sqWv[:, :, 0, :], sqWv[:, :, 1, :])
                nc.gpsimd.tensor_add(
                    S9v[:, 0:4:3, :], sdtv[:, 0:2, :], sqWv[:, 0:2, 2, :]
                )
                nc.gpsimd.tensor_add(S9v[:, 8, :], sdtv[:, 2, :], sqWv[:, 2, 2, :])
                # pp: row-pair products (01, 02, 12)
                pp = pmat.tile([P, 9 * F1], F16, tag="pp", name="pp")
                ppv = pp[:].rearrange("p (g k f) -> p g k f", g=3, k=3)
                nc.vector.tensor_mul(
                    ppv[:, 0:2, :, :],
                    W9v4[:, 0, :, :].unsqueeze(1).to_broadcast((P, 2, 3, F1)),
                    W9v4[:, 1:3, :, :],
                )
                nc.vector.tensor_mul(
                    ppv[:, 2, :, :], W9v4[:, 1, :, :], W9v4[:, 2, :, :]
                )
                q3 = psq.tile([P, 3 * F1], F16, tag="q3", name="q3")
                q3v = q3[:].rearrange("p (g f) -> p g f", g=3)
                nc.gpsimd.tensor_add(q3v, ppv[:, :, 0, :], ppv[:, :, 1, :])
                nc.gpsimd.tensor_add(
                    S9v[:, 1:3, :], q3v[:, 0:2, :], ppv[:, 0:2, 2, :]
                )
                nc.gpsimd.tensor_add(S9v[:, 5, :], q3v[:, 2, :], ppv[:, 2, 2, :])

                # M = R @ S (DVE muls, Pool final add into out tile)
                srows = [S9v[:, 0:3, :], S9v[:, 1:7:2, :], S9v[:, 2:9:3, :]]
                otv = ot[:].rearrange("p (i j f) -> p i j f", i=3, j=3)[
                    :, :, :, osl
                ]

                def colb(k):
                    return (
                        R9v[:, k : k + 7 : 3, :]
                        .unsqueeze(2)
                        .to_broadcast((P, 3, 3, F1))
                    )

                def rowb(sr):
                    return sr.unsqueeze(1).to_broadcast((P, 3, 3, F1))

                mp1 = pmat.tile([P, 9 * F1], F16, tag="pp", name="mp1")
                mp1v = mp1[:].rearrange("p (i j f) -> p i j f", i=3, j=3)
                nc.vector.tensor_mul(mp1v, colb(0), rowb(srows[0]))
                mp2 = pmat.tile([P, 9 * F1], F16, tag="sqW", name="mp2")
                mp2v = mp2[:].rearrange("p (i j f) -> p i j f", i=3, j=3)
                nc.vector.tensor_mul(mp2v, colb(1), rowb(srows[1]))
                ms = pms.tile([P, 9 * F1], F16, tag="ms", name="ms")
                msv = ms[:].rearrange("p (i j f) -> p i j f", i=3, j=3)
                (nc.vector if b % 2 == 0 else nc.gpsimd).tensor_add(
                    msv, mp1v, mp2v
                )
                mp3 = pmat.tile([P, 9 * F1], F16, tag="W9", name="mp3")
                mp3v = mp3[:].rearrange("p (i j f) -> p i j f", i=3, j=3)
                nc.vector.tensor_mul(mp3v, colb(2), rowb(srows[2]))
                nc.gpsimd.tensor_add(otv, msv, mp3v)

            for q in range(NQ):
                st = chain(q)
                for pr in range(2):  # block-pairs per quad
                    ot = pout.tile([P, 18 * F1], F16, tag="ot", name="ot")
                    for j in range(2):
                        b = 2 * pr + j
                        osl = slice(j * F1, (j + 1) * F1)
                        build(st, b, ot, osl, last=(q == NQ - 1 and b == 3))
                    osl2 = slice(pr * 2 * F1, (pr + 1) * 2 * F1)
                    nc.sync.dma_start(
                        out=vout[q][:, :, osl2],
                        in_=ot[:].rearrange("p (k f) -> p k f", k=9),
                    )

    _split_multi_waits(nc)
    return nc


# ----------------------------------------------------------------------------
# host-side execution
# ----------------------------------------------------------------------------
_CACHE = {}


def _get_runner():
    if "runner" in _CACHE:
        return _CACHE["runner"]
    import jax
    from jax.sharding import Mesh, PartitionSpec
    from jax.experimental.shard_map import shard_map
    from concourse.bass2jax import (
        _bass_exec_p,
        install_neuronx_cc_hook,
        partition_id_tensor,
    )

    nc = build_module()
    install_neuronx_cc_hook()
    partition_name = nc.partition_id_tensor.name if nc.partition_id_tensor else None
    in_names, out_names, out_avals, zero_outs = [], [], [], []
    for alloc in nc.m.functions[0].allocations:
        if not isinstance(alloc, mybir.MemoryLocationSet):
            continue
        name = alloc.memorylocations[0].name
        if alloc.kind == "ExternalInput":
            if name != partition_name:
                in_names.append(name)
        elif alloc.kind == "ExternalOutput":
            shape = tuple(alloc.tensor_shape)
            dtype = mybir.dt.np(alloc.dtype)
            out_names.append(name)
            out_avals.append(jax.core.ShapedArray(shape, dtype))
            zero_outs.append(np.zeros(shape, dtype))
    n_params = len(in_names)
    all_in_names = in_names + out_names + (
        [partition_name] if partition_name else []
    )

    def _body(*args):
        operands = list(args)
        if partition_name is not None:
            operands.append(partition_id_tensor())
        outs = _bass_exec_p.bind(
            *operands,
            out_avals=tuple(out_avals),
            in_names=tuple(all_in_names),
            out_names=tuple(out_names),
            lowering_input_output_aliases=(),
            sim_require_finite=True,
            sim_require_nnan=True,
            nc=nc,
        )
        return tuple(outs)

    devices = jax.devices()[:N_CORES]
    mesh = Mesh(np.asarray(devices), ("core",))
    n_outs = len(out_names)
    jf = jax.jit(
        shard_map(
            _body,
            mesh=mesh,
            in_specs=(PartitionSpec("core"),) * (n_params + n_outs),
            out_specs=(PartitionSpec("core"),) * n_outs,
            check_rep=False,
        ),
        donate_argnums=tuple(range(n_params, n_params + n_outs)),
        keep_unused=True,
    )
    _CACHE["runner"] = (jf, in_names, out_names, zero_outs)
    return _CACHE["runner"]


def kernel(trans, rotat, scal_dir, scal):
    jf, in_names, out_names, zero_outs = _get_runner()
    assert in_names == ["in9"], in_names

    # fp16 planar marshalling: planes (rx ry rz ux uy uz sx sy sz) per core
    a = np.empty((N_CORES, 9, E), dtype=np.float16)
    buf = np.ones((BPAD, 3), dtype=np.float16)
    for i, src in enumerate((rotat, scal_dir, scal)):
        buf[:B] = src
        if i > 0:
            buf[B:] = 1.0
        a[:, 3 * i : 3 * i + 3, :] = buf.reshape(N_CORES, E, 3).transpose(0, 2, 1)

    in9_host = a.reshape(N_CORES * 9, E)
    zeros = [
        np.zeros((N_CORES * z.shape[0], *z.shape[1:]), z.dtype) for z in zero_outs
    ]
    outs = jf(in9_host, *zeros)
    o = np.asarray(outs[0]).reshape(N_CORES, 9, E)
    m = o.transpose(0, 2, 1).reshape(BPAD, 9)[:B].astype(np.float32)
    out = np.empty((B, 3, 4), dtype=np.float32)
    out[:, :, :3] = m.reshape(B, 3, 3)
    out[:, :, 3] = trans
    return out


if __name__ == "__main__":
    rng = np.random.default_rng(0)
    ins = {
        "trans": rng.normal(size=(B, 3)).astype(np.float32),
        "rotat": rng.normal(size=(B, 3)).astype(np.float32),
        "scal_dir": rng.normal(size=(B, 3)).astype(np.float32),
        "scal": rng.normal(size=(B, 3)).astype(np.float32),
    }
    out = kernel(**ins)
    print(out.shape, out.dtype)
